# revision 1
# baseline (speedup 1.0000x reference)
"""Trainium2 Bass kernel for nn_BarrierPolicy (CBF-QP safety filter).

Data-parallel over batch: 8 cores x 32768 samples.
Phase A (per 2048-sample tile): load x in "xview" layout, PE-transpose to
"SP2" (stacked pack-2) layout, run the 3-layer MLP + dynamics matmuls on the
tensor engine, transpose results back to xview.
Phase B (full core): Kiwiel variable-fixing active-set solve of the
per-sample box-QP dual (5 iterations + closed-form finish), then
u = clip(-p + lam*g).

Layouts (per tile of 2048 samples):
  xview: SBUF (128, 128): partition r, col 16b+8s0+j <-> sample 256b+2r+s0, coord j
  SP2  : transpose of xview: partition 16b+8s0+j, col r
  padded-pair psum (for 16-row matmul outs, 32-align rule): chunk b=2q+h at
  partitions [32q,32q+16), free-slot h.
  slot : per-sample scalars (128, 16): partition r, col 2b+s0
"""
import numpy as np

B_FULL, N = 262144, 8
NCORES = 8
S = B_FULL // NCORES          # 32768 samples per core
TILE = 2048
NT = S // TILE                # 16 tiles
NSLOT = S // 128              # 256 slot cols per core
T_KIWIEL = 5
LAMCAP = float(2.0 ** 40)
EPS = 1e-12

_CACHE = {}

_CSHAPES = dict(TL2=(128, 128), TL3px=(64, 16), TL3a=(128, 2),
                TDA=(128, 128), TDG=(128, 128), ID128=(128, 128),
                B1v=(128, 1), B2v=(128, 1), B31e=(128, 1), B32e=(128, 1),
                **{f"TL1E{b}": (128, 128) for b in range(8)})


def _consts(W1, b1, W21, b21, W22, b22, W31, b31, W32, b32, A, G):
    f32 = np.float32
    out = {}
    for b in range(8):
        T = np.zeros((128, 128), f32)
        for s0 in range(2):
            T[16 * b + 8 * s0:16 * b + 8 * s0 + 8, 64 * s0:64 * s0 + 64] = W1
        out[f"TL1E{b}"] = T
    TL2 = np.zeros((128, 128), f32)
    for s0 in range(2):
        TL2[64 * s0:64 * s0 + 64, 32 * s0:32 * s0 + 32] = W21
        TL2[64 * s0:64 * s0 + 64, 64 + 32 * s0:64 + 32 * s0 + 32] = W22
    TL3px = np.zeros((64, 16), f32)
    for s0 in range(2):
        TL3px[32 * s0:32 * s0 + 32, 8 * s0:8 * s0 + 8] = W31
    TL3a = np.zeros((128, 2), f32)          # used as slice [64:128)
    for s0 in range(2):
        TL3a[64 + 32 * s0:64 + 32 * s0 + 32, s0:s0 + 1] = W32
    TDA = np.kron(np.eye(16, dtype=f32), A.T.astype(f32))         # out = A x
    TDG = np.kron(np.eye(16, dtype=f32), (-2.0 * G).astype(f32))  # out = -2 G^T x
    ID128 = np.eye(128, dtype=f32)
    B1v = np.concatenate([b1, b1]).reshape(128, 1).astype(f32)
    B2v = np.concatenate([b21, b21, b22, b22]).reshape(128, 1).astype(f32)
    B31e = np.zeros((128, 1), f32)          # bias for padded px evac (3 bases)
    for m in range(3):
        for s0 in range(2):
            B31e[32 * m + 8 * s0:32 * m + 8 * s0 + 8, 0] = b31
    B32e = np.full((128, 1), float(b32[0]), f32)
    out.update(TL2=TL2, TL3px=TL3px, TL3a=TL3a, TDA=TDA, TDG=TDG, ID128=ID128,
               B1v=B1v, B2v=B2v, B31e=B31e, B32e=B32e)
    return out


def build_kernel(nc, tc, x_d, u_d, cds):
    from concourse import mybir
    f32 = mybir.dt.float32
    AL = mybir.AluOpType
    AF = mybir.ActivationFunctionType
    XL = mybir.AxisListType.X

    with (
        tc.tile_pool(name="const", bufs=1) as cpool,
        tc.tile_pool(name="pers", bufs=1) as pers,
        tc.tile_pool(name="work", bufs=2) as work,
        tc.tile_pool(name="psA", bufs=1, space="PSUM") as psA,
        tc.tile_pool(name="psB", bufs=1, space="PSUM") as psB,
    ):
        C = {k: cpool.tile(list(v), f32, tag=k, name=k) for k, v in _CSHAPES.items()}
        for k in _CSHAPES:
            nc.sync.dma_start(C[k][:], cds[k][:])

        FC = S // 16   # 2048 xview cols per core
        def fc_tile(tag):
            return pers.tile([128, FC], f32, tag=tag, name=tag)
        x_xv, p_xv, g_xv = fc_tile("x_xv"), fc_tile("p_xv"), fc_tile("g_xv")
        gt_xv, pt_xv, q_xv = fc_tile("gt_xv"), fc_tile("pt_xv"), fc_tile("q_xv")
        zt_xv, mm_xv = fc_tile("zt_xv"), fc_tile("mm_xv")
        sc1, sc2 = fc_tile("sc1"), fc_tile("sc2")
        def sl_tile(tag):
            return pers.tile([128, NSLOT], f32, tag=tag, name=tag)
        alpha4, lfhx, sxx = sl_tile("alpha4"), sl_tile("lfhx"), sl_tile("sxx")
        c0s, viol, infs = sl_tile("c0s"), sl_tile("viol"), sl_tile("infs")
        nums, dens, lams = sl_tile("nums"), sl_tile("dens"), sl_tile("lams")
        t1s, t2s, nus, bvs = sl_tile("t1s"), sl_tile("t2s"), sl_tile("nus"), sl_tile("bvs")

        # ---------------- Phase A ----------------
        for t in range(NT):
            cs = slice(128 * t, 128 * t + 128)
            ss = slice(16 * t, 16 * t + 16)
            nc.sync.dma_start(
                x_xv[:, cs].rearrange("p (b s j) -> p b s j", b=8, s=2, j=8),
                x_d[t * TILE:(t + 1) * TILE, :].rearrange(
                    "(b r s) j -> r b s j", b=8, r=128, s=2))
            TP = psA.tile([128, 3, 128], f32, tag="TP", name="TP")
            nc.tensor.transpose(TP[:, 0, :], x_xv[:, cs], C["ID128"][:])
            xsp2 = work.tile([128, 128], f32, tag="xsp2", name="xsp2")
            nc.vector.tensor_copy(xsp2[:], TP[:, 0, :])

            h1P = psA.tile([128, 4, 128], f32, tag="h1P", name="h1P")
            x2P = psA.tile([128, 4, 128], f32, tag="x2P", name="x2P")
            LPx = psA.tile([128, 3, 128], f32, tag="LPx", name="LPx")
            alP = psA.tile([128, 3, 128], f32, tag="alP", name="alP")
            h1 = work.tile([128, 8, 128], f32, tag="h1", name="h1")
            x2 = work.tile([128, 8, 128], f32, tag="x2", name="x2")
            pxe = work.tile([128, 3, 128], f32, tag="pxe", name="pxe")
            asle = work.tile([128, 3, 128], f32, tag="asle", name="asle")

            for half in range(2):
                for bi in range(4):
                    b = 4 * half + bi
                    nc.tensor.matmul(h1P[:, bi, :], C[f"TL1E{b}"][:], xsp2[:])
                for bi in range(4):
                    b = 4 * half + bi
                    nc.scalar.activation(h1[:, b, :], h1P[:, bi, :], AF.Relu,
                                         bias=C["B1v"][:])
                for bi in range(4):
                    b = 4 * half + bi
                    nc.tensor.matmul(x2P[:, bi, :], C["TL2"][:], h1[:, b, :])
                for bi in range(4):
                    b = 4 * half + bi
                    nc.scalar.activation(x2[:, b, :], x2P[:, bi, :], AF.Relu,
                                         bias=C["B2v"][:])
                for bi in range(4):
                    b = 4 * half + bi
                    m3, k3 = b % 3, b // 3
                    nc.tensor.matmul(LPx[32 * m3:32 * m3 + 16, k3, :],
                                     C["TL3px"][:], x2[0:64, b, :])
                    nc.tensor.matmul(alP[32 * m3:32 * m3 + 2, k3, :],
                                     C["TL3a"][64:128, :], x2[64:128, b, :])
            nc.gpsimd.memset(pxe[:], 0.0)
            nc.gpsimd.memset(asle[:], 0.0)
            for m in range(3):
                kk = 3 if m < 2 else 2
                nc.vector.tensor_scalar(pxe[32 * m:32 * m + 16, 0:kk, :],
                                        LPx[32 * m:32 * m + 16, 0:kk, :],
                                        C["B31e"][32 * m:32 * m + 16, :], None,
                                        AL.add)
                nc.scalar.activation(asle[32 * m:32 * m + 2, 0:kk, :],
                                     alP[32 * m:32 * m + 2, 0:kk, :], AF.Sigmoid,
                                     bias=C["B32e"][32 * m:32 * m + 2, :])

            nc.tensor.matmul(TP[:, 1, :], C["TDA"][:], xsp2[:])
            nc.tensor.matmul(TP[:, 2, :], C["TDG"][:], xsp2[:])
            axs = work.tile([128, 128], f32, tag="axs", name="axs")
            gsp2 = work.tile([128, 128], f32, tag="gsp2", name="gsp2")
            nc.vector.tensor_copy(axs[:], TP[:, 1, :])
            nc.scalar.activation(gsp2[:], TP[:, 2, :], AF.Copy)

            # transposes back to xview
            trP = psB.tile([128, 2, 128], f32, tag="trP", name="trP")
            nc.tensor.transpose(trP[:, 0, :], gsp2[:], C["ID128"][:])
            nc.tensor.transpose(trP[:, 1, :], axs[:], C["ID128"][:])
            nc.scalar.activation(g_xv[:, cs], trP[:, 0, :], AF.Copy)
            prodA = work.tile([128, 128], f32, tag="prodA", name="prodA")
            nc.vector.scalar_tensor_tensor(prodA[:], trP[:, 1, :], -2.0,
                                           x_xv[:, cs], AL.mult, AL.mult)
            nc.vector.tensor_reduce(lfhx[:, ss],
                                    prodA[:].rearrange("p (c j) -> p c j", j=8),
                                    XL, AL.add)
            sqx = work.tile([128, 128], f32, tag="sqx", name="sqx")
            nc.scalar.activation(sqx[:], x_xv[:, cs], AF.Square)
            nc.vector.tensor_reduce(sxx[:, ss],
                                    sqx[:].rearrange("p (c j) -> p c j", j=8),
                                    XL, AL.add)

            pxtP = psB.tile([128, 3, 128], f32, tag="pxtP", name="pxtP")
            altP = psB.tile([128, 3, 128], f32, tag="altP", name="altP")
            for k in range(3):
                nc.tensor.transpose(pxtP[:, k, :], pxe[:, k, :], C["ID128"][:])
                nc.tensor.transpose(altP[:, k, :], asle[:, k, :], C["ID128"][:])
            for k in range(3):
                nm = 3 if k < 2 else 2
                dstp = p_xv[:, cs].rearrange("p (b s j) -> p b s j",
                                             b=8, s=2, j=8)[:, 3 * k:3 * k + nm, :, :]
                srcp = pxtP[:, k, :].rearrange("p (m g s j) -> p m g s j",
                                               m=4, g=2, s=2, j=8)[:, 0:nm, 0, :, :]
                nc.vector.tensor_copy(dstp, srcp)
                dsta = alpha4[:, ss].rearrange("p (b s) -> p b s",
                                               b=8, s=2)[:, 3 * k:3 * k + nm, :]
                srca = altP[:, k, :].rearrange("p (m g) -> p m g",
                                               m=4, g=32)[:, 0:nm, 0:2]
                nc.vector.tensor_copy(dsta, srca)

        # ---------------- Phase B ----------------
        x3 = lambda ap: ap.rearrange("p (c j) -> p c j", j=8)
        bc = lambda ap: ap.broadcast_to((128, NSLOT, 8))
        V, GP, SC = nc.vector, nc.gpsimd, nc.scalar

        GP.tensor_scalar(alpha4[:], alpha4[:], 4.0, None, AL.mult)
        GP.tensor_scalar(t1s[:], sxx[:], -1.0, 16.0, AL.mult, AL.add)
        V.tensor_tensor(t2s[:], alpha4[:], t1s[:], AL.mult)
        V.tensor_tensor(c0s[:], t2s[:], lfhx[:], AL.add)

        SC.sign(sc1[:], g_xv[:])                                  # sigma
        V.tensor_tensor(pt_xv[:], sc1[:], p_xv[:], AL.mult)       # pt
        GP.tensor_scalar(zt_xv[:], pt_xv[:], -1.0, None, AL.mult)  # zt0
        SC.activation(gt_xv[:], g_xv[:], AF.Abs)
        SC.activation(q_xv[:], g_xv[:], AF.Square)
        V.memset(mm_xv[:], 1.0)

        V.tensor_scalar(sc2[:], p_xv[:], -1.0, 1.0, AL.mult, AL.min)
        V.tensor_scalar(sc2[:], sc2[:], -1.0, None, AL.max)
        V.tensor_tensor(sc2[:], g_xv[:], sc2[:], AL.mult)
        V.tensor_reduce(t1s[:], x3(sc2[:]), XL, AL.add)
        V.tensor_tensor(t1s[:], c0s[:], t1s[:], AL.add)
        GP.tensor_scalar(viol[:], t1s[:], 0.0, None, AL.is_lt)
        V.tensor_reduce(t2s[:], x3(gt_xv[:]), XL, AL.add)
        V.tensor_tensor(t2s[:], c0s[:], t2s[:], AL.add)
        GP.tensor_scalar(infs[:], t2s[:], 0.0, None, AL.is_lt)
        V.tensor_tensor(infs[:], infs[:], viol[:], AL.mult)

        def calc_num_den():
            V.tensor_tensor(sc1[:], gt_xv[:], zt_xv[:], AL.mult)
            V.tensor_reduce(nums[:], x3(sc1[:]), XL, AL.add)
            V.tensor_tensor(nums[:], c0s[:], nums[:], AL.add)
            GP.tensor_tensor(sc2[:], q_xv[:], mm_xv[:], AL.mult)
            V.tensor_reduce(dens[:], x3(sc2[:]), XL, AL.add)

        def calc_lam():
            GP.tensor_scalar(t1s[:], dens[:], EPS, None, AL.add)
            V.reciprocal(t2s[:], t1s[:])
            V.scalar_tensor_tensor(lams[:], nums[:], -1.0, t2s[:], AL.mult, AL.mult)
            V.tensor_tensor(lams[:], lams[:], viol[:], AL.mult)

        calc_num_den()
        for _ in range(T_KIWIEL):
            calc_lam()
            V.tensor_tensor(x3(sc1[:]), bc(lams[:]), x3(gt_xv[:]), AL.mult)
            V.tensor_tensor(sc1[:], sc1[:], pt_xv[:], AL.subtract)   # ur
            V.tensor_scalar(sc2[:], sc1[:], 1.0, -1.0, AL.min, AL.max)
            V.tensor_tensor(sc2[:], gt_xv[:], sc2[:], AL.mult)
            V.tensor_reduce(t1s[:], x3(sc2[:]), XL, AL.add)
            V.tensor_tensor(t1s[:], c0s[:], t1s[:], AL.add)          # c
            GP.tensor_scalar(nus[:], t1s[:], 0.0, None, AL.is_lt)    # needup
            GP.tensor_scalar(bvs[:], nus[:], 2.0, -1.0, AL.mult, AL.add)
            # fix = M * 1{B*ur >= 1}  (== M*(NU*m1 + (1-NU)*m2))
            V.tensor_tensor(x3(sc2[:]), bc(bvs[:]), x3(sc1[:]), AL.mult)
            V.tensor_scalar(sc2[:], sc2[:], 1.0, None, AL.is_ge)
            V.tensor_tensor(sc2[:], sc2[:], mm_xv[:], AL.mult)       # fix
            GP.tensor_tensor(x3(sc1[:]), bc(bvs[:]), x3(zt_xv[:]), AL.subtract)
            V.tensor_tensor(sc1[:], sc2[:], sc1[:], AL.mult)
            V.tensor_tensor(zt_xv[:], zt_xv[:], sc1[:], AL.add)
            GP.tensor_tensor(mm_xv[:], mm_xv[:], sc2[:], AL.subtract)
            calc_num_den()
        calc_lam()
        GP.tensor_scalar(t1s[:], lams[:], -1.0, LAMCAP, AL.mult, AL.add)
        V.tensor_tensor(t1s[:], t1s[:], infs[:], AL.mult)
        V.tensor_tensor(lams[:], lams[:], t1s[:], AL.add)
        V.tensor_tensor(x3(sc1[:]), bc(lams[:]), x3(g_xv[:]), AL.mult)
        V.tensor_tensor(sc1[:], sc1[:], p_xv[:], AL.subtract)
        V.tensor_scalar(sc1[:], sc1[:], 1.0, -1.0, AL.min, AL.max)
        for t in range(NT):
            nc.sync.dma_start(
                u_d[t * TILE:(t + 1) * TILE, :].rearrange(
                    "(b r s) j -> r b s j", b=8, r=128, s=2),
                sc1[:, 128 * t:128 * t + 128].rearrange(
                    "p (b s j) -> p b s j", b=8, s=2, j=8))


def _build():
    from concourse import bacc, mybir
    from concourse import tile as tile_mod
    from concourse._compat import axon_active
    f32 = mybir.dt.float32
    nc = bacc.Bacc("TRN2", target_bir_lowering=False,
                   debug=not axon_active(), num_devices=NCORES)
    x_d = nc.dram_tensor("x", [S, N], f32, kind="ExternalInput").ap()
    u_d = nc.dram_tensor("u", [S, N], f32, kind="ExternalOutput").ap()
    cds = {k: nc.dram_tensor(k, list(v), f32, kind="ExternalInput").ap()
           for k, v in _CSHAPES.items()}
    with tile_mod.TileContext(nc) as tc:
        build_kernel(nc, tc, x_d, u_d, cds)
    nc.compile()
    return nc


def kernel(x, W1, b1, W21, b21, W22, b22, W31, b31, W32, b32, A, G, mean, std):
    from concourse.bass_utils import run_bass_kernel_spmd
    f32 = np.float32
    x = np.asarray(x, f32)
    x0 = (x * np.asarray(std, f32) + np.asarray(mean, f32)).astype(f32)

    consts = _consts(np.asarray(W1, f32), np.asarray(b1, f32), np.asarray(W21, f32),
                     np.asarray(b21, f32), np.asarray(W22, f32), np.asarray(b22, f32),
                     np.asarray(W31, f32), np.asarray(b31, f32), np.asarray(W32, f32),
                     np.asarray(b32, f32), np.asarray(A, f32), np.asarray(G, f32))
    if "nc" not in _CACHE:
        _CACHE["nc"] = _build()
    nc = _CACHE["nc"]

    in_maps = []
    for c in range(NCORES):
        m = {"x": np.ascontiguousarray(x0[c * S:(c + 1) * S])}
        m.update(consts)
        in_maps.append(m)
    res = run_bass_kernel_spmd(nc, in_maps, list(range(NCORES)))
    out = np.concatenate([np.asarray(res.results[c]["u"]) for c in range(NCORES)],
                         axis=0)
    return out.astype(f32)



# revision 10
# speedup vs baseline: 2.2633x; 2.2633x over previous
"""Trainium2 Bass kernel for nn_BarrierPolicy (CBF-QP safety filter).

Data-parallel over batch: 8 cores x 32768 samples.
Phase A (per 2048-sample tile): load x in "xview" layout, PE-transpose to
"SP2" (stacked pack-2) layout, run the 3-layer MLP + dynamics matmuls on the
tensor engine in bf16 (1 cycle/row), transpose results back to xview.
Bias-add for px and the sigmoid for alpha are deferred to xview where they
are single wide ops instead of many narrow ones.
Phase B (per 1024-col chunk, 2 chunks): Kiwiel variable-fixing active-set
solve of the per-sample box-QP dual (3 iterations + closed-form finish) in
bf16 elementwise / f32 slot math, then u = clip(-p + lam*g) in f32.
Broadcast of per-sample scalars over the 8 coords is materialized by the
scalar (ACT) engine; slot math mostly on GPSIMD to keep DVE for the wide
bf16 elementwise ops.

Layouts (per tile of 2048 samples):
  xview: SBUF (128, 128): partition r, col 16b+8s0+j <-> sample 256b+2r+s0, coord j
  SP2  : transpose of xview: partition 16b+8s0+j, col r
  padded psum (for 16-row matmul outs, 32-align rule): block b at partitions
  [32(b%3), 32(b%3)+16), free-slot b//3.
  slot : per-sample scalars (128, 256): partition r, col 2b+s0 per tile
"""
import numpy as np

B_FULL, N = 262144, 8
NCORES = 8
S = B_FULL // NCORES          # 32768 samples per core
TILE = 2048
NT = S // TILE                # 16 tiles
NSLOT = S // 128              # 256 slot cols per core
NCH = 2                       # phase-B chunks
TPC = NT // NCH               # tiles per chunk
FC = S // 16                  # 2048 xview cols per core
T_KIWIEL = 3
LAMCAP = float(2.0 ** 40)
EPS = 1e-12

_CACHE = {}

_CSHAPES_BF = dict(TL2=(128, 128), TL3px=(64, 16), TL3a=(128, 2),
                   TDA=(128, 128), TDG=(128, 128), ID128H=(128, 128),
                   B31X=(128, 128),
                   **{f"TL1E{b}": (128, 128) for b in range(8)})
_CSHAPES_F32 = dict(ID128=(128, 128), B1v=(128, 1), B2v=(128, 1),
                    B32s=(128, 1))


def _consts(W1, b1, W21, b21, W22, b22, W31, b31, W32, b32, A, G):
    import ml_dtypes
    f32 = np.float32
    bf = ml_dtypes.bfloat16
    out = {}
    for b in range(8):
        T = np.zeros((128, 128), f32)
        for s0 in range(2):
            T[16 * b + 8 * s0:16 * b + 8 * s0 + 8, 64 * s0:64 * s0 + 64] = W1
        out[f"TL1E{b}"] = T.astype(bf)
    TL2 = np.zeros((128, 128), f32)
    for s0 in range(2):
        TL2[64 * s0:64 * s0 + 64, 32 * s0:32 * s0 + 32] = W21
        TL2[64 * s0:64 * s0 + 64, 64 + 32 * s0:64 + 32 * s0 + 32] = W22
    TL3px = np.zeros((64, 16), f32)
    for s0 in range(2):
        TL3px[32 * s0:32 * s0 + 32, 8 * s0:8 * s0 + 8] = W31
    TL3a = np.zeros((128, 2), f32)          # used as slice [64:128)
    for s0 in range(2):
        TL3a[64 + 32 * s0:64 + 32 * s0 + 32, s0:s0 + 1] = W32
    TDA = np.kron(np.eye(16, dtype=f32), A.T.astype(f32))         # out = A x
    TDG = np.kron(np.eye(16, dtype=f32), (-2.0 * G).astype(f32))  # out = -2 G^T x
    # xview b31 bias tile: col 16b+8s0+j -> b31[j]
    B31X = np.tile(b31.astype(f32), 16)[None, :].repeat(128, 0)
    out.update(TL2=TL2.astype(bf), TL3px=TL3px.astype(bf), TL3a=TL3a.astype(bf),
               TDA=TDA.astype(bf), TDG=TDG.astype(bf),
               ID128H=np.eye(128, dtype=f32).astype(bf),
               B31X=B31X.astype(bf))
    out["ID128"] = np.eye(128, dtype=f32)
    out["B1v"] = np.concatenate([b1, b1]).reshape(128, 1).astype(f32)
    out["B2v"] = np.concatenate([b21, b21, b22, b22]).reshape(128, 1).astype(f32)
    out["B32s"] = np.full((128, 1), float(b32[0]), f32)
    return out


def build_kernel(nc, tc, x_d, u_d, cds):
    from concourse import mybir
    f32 = mybir.dt.float32
    f32r = mybir.dt.float32r
    bf16 = mybir.dt.bfloat16
    AL = mybir.AluOpType
    AF = mybir.ActivationFunctionType
    XL = mybir.AxisListType.X
    V, GP, SC = nc.vector, nc.gpsimd, nc.scalar

    with (
        tc.tile_pool(name="const", bufs=1) as cpool,
        tc.tile_pool(name="pers", bufs=1) as pers,
        tc.tile_pool(name="work", bufs=2) as work,
        tc.tile_pool(name="psA", bufs=1, space="PSUM") as psA,
        tc.tile_pool(name="psB", bufs=1, space="PSUM") as psB,
    ):
        C = {}
        for k, v in _CSHAPES_BF.items():
            C[k] = cpool.tile(list(v), bf16, tag=k, name=k)
        for k, v in _CSHAPES_F32.items():
            C[k] = cpool.tile(list(v), f32, tag=k, name=k)
        for k in list(_CSHAPES_BF) + list(_CSHAPES_F32):
            nc.sync.dma_start(C[k][:], cds[k][:])

        def fc_f32(tag):
            return pers.tile([128, FC], f32, tag=tag, name=tag)

        def fc_bf(tag):
            return pers.tile([128, FC], bf16, tag=tag, name=tag)

        def sl_tile(tag):
            return pers.tile([128, NSLOT], f32, tag=tag, name=tag)

        x_xv, u32 = fc_f32("x_xv"), fc_f32("u32")
        p_xv, g_xv = fc_bf("p_xv"), fc_bf("g_xv")
        gt_xv, pt_xv, q_xv = fc_bf("gt_xv"), fc_bf("pt_xv"), fc_bf("q_xv")
        zt_xv, mm_xv = fc_bf("zt_xv"), fc_bf("mm_xv")
        sc1, sc2, sc3 = fc_bf("sc1"), fc_bf("sc2"), fc_bf("sc3")
        lbc, bvbc = fc_bf("lbc"), fc_bf("bvbc")
        araw = sl_tile("araw")
        alpha4, lfhx, sxx = sl_tile("alpha4"), sl_tile("lfhx"), sl_tile("sxx")
        c0s, viol, nviol, infs = (sl_tile("c0s"), sl_tile("viol"),
                                  sl_tile("nviol"), sl_tile("infs"))
        nums, dens, lams = sl_tile("nums"), sl_tile("dens"), sl_tile("lams")
        t1s, t2s = sl_tile("t1s"), sl_tile("t2s")

        # ---------------- Phase A ----------------
        # px/alpha matmul outs leave pad regions unwritten; zero once so the
        # full-tile evac copies and transposes never see uninitialized PSUM.
        LPx = psA.tile([128, 3, 128], f32, tag="LPx", name="LPx")
        alP = psA.tile([128, 3, 128], f32, tag="alP", name="alP")
        V.memset(LPx[:], 0.0)
        V.memset(alP[:], 0.0)
        for t in range(NT):
            cs = slice(128 * t, 128 * t + 128)
            ss = slice(16 * t, 16 * t + 16)
            nc.sync.dma_start(
                x_xv[:, cs].rearrange("p (b s j) -> p b s j", b=8, s=2, j=8),
                x_d[t * TILE:(t + 1) * TILE, :].rearrange(
                    "(b r s) j -> r b s j", b=8, r=128, s=2))
            TP = psA.tile([128, 3, 128], f32, tag="TP", name="TP")
            nc.tensor.transpose(TP[:, 0, :], x_xv[:, cs], C["ID128"][:])
            xsp2 = work.tile([128, 128], bf16, tag="xsp2", name="xsp2")
            V.tensor_copy(xsp2[:], TP[:, 0, :])

            h1P = psA.tile([128, 4, 128], f32, tag="h1P", name="h1P")
            x2P = psA.tile([128, 4, 128], f32, tag="x2P", name="x2P")
            h1 = work.tile([128, 8, 128], bf16, tag="h1", name="h1")
            x2 = work.tile([128, 8, 128], bf16, tag="x2", name="x2")

            for half in range(2):
                hs = slice(4 * half, 4 * half + 4)
                for bi in range(4):
                    b = 4 * half + bi
                    nc.tensor.matmul(h1P[:, bi, :], C[f"TL1E{b}"][:], xsp2[:])
                # relu + bias evac, one wide op per half (PSUM: DVE/ACT only)
                SC.activation(h1[:, hs, :], h1P[:], AF.Relu, bias=C["B1v"][:])
                for bi in range(4):
                    b = 4 * half + bi
                    nc.tensor.matmul(x2P[:, bi, :], C["TL2"][:], h1[:, b, :])
                if half == 0:
                    V.tensor_scalar(x2[:, hs, :], x2P[:], C["B2v"][:], 0.0,
                                    AL.add, AL.max)
                else:
                    SC.activation(x2[:, hs, :], x2P[:], AF.Relu, bias=C["B2v"][:])
                for bi in range(4):
                    b = 4 * half + bi
                    m3, k3 = b % 3, b // 3
                    nc.tensor.matmul(LPx[32 * m3:32 * m3 + 16, k3, :],
                                     C["TL3px"][:], x2[0:64, b, :])
                    nc.tensor.matmul(alP[32 * m3:32 * m3 + 2, k3, :],
                                     C["TL3a"][64:128, :], x2[64:128, b, :])

            # raw px / alpha evac (bias+sigmoid deferred to xview)
            pxe = work.tile([128, 3, 128], bf16, tag="pxe", name="pxe")
            asle = work.tile([128, 3, 128], bf16, tag="asle", name="asle")
            SC.activation(pxe[:], LPx[:], AF.Copy)
            SC.activation(asle[:], alP[:], AF.Copy)

            nc.tensor.matmul(TP[:, 1, :], C["TDA"][:], xsp2[:])
            nc.tensor.matmul(TP[:, 2, :], C["TDG"][:], xsp2[:])
            axs = work.tile([128, 128], bf16, tag="axs", name="axs")
            gsp2 = work.tile([128, 128], bf16, tag="gsp2", name="gsp2")
            V.tensor_copy(axs[:], TP[:, 1, :])
            V.tensor_copy(gsp2[:], TP[:, 2, :])

            # transposes back to xview
            trP = psB.tile([128, 2, 128], bf16, tag="trP", name="trP")
            nc.tensor.transpose(trP[:, 0, :], gsp2[:], C["ID128H"][:])
            nc.tensor.transpose(trP[:, 1, :], axs[:], C["ID128H"][:])
            V.tensor_copy(g_xv[:, cs], trP[:, 0, :])
            prodA = work.tile([128, 128], f32, tag="prodA", name="prodA")
            V.scalar_tensor_tensor(prodA[:], trP[:, 1, :], -2.0,
                                   x_xv[:, cs], AL.mult, AL.mult)
            V.tensor_reduce(lfhx[:, ss],
                            prodA[:].rearrange("p (c j) -> p c j", j=8),
                            XL, AL.add)
            sqx = work.tile([128, 128], f32, tag="sqx", name="sqx")
            GP.tensor_tensor(sqx[:], x_xv[:, cs], x_xv[:, cs], AL.mult)
            V.tensor_reduce(sxx[:, ss],
                            sqx[:].rearrange("p (c j) -> p c j", j=8),
                            XL, AL.add)

            pxtP = psB.tile([128, 3, 128], bf16, tag="pxtP", name="pxtP")
            altP = psB.tile([128, 3, 128], bf16, tag="altP", name="altP")
            for k in range(3):
                nc.tensor.transpose(pxtP[:, k, :], pxe[:, k, :], C["ID128H"][:])
                nc.tensor.transpose(altP[:, k, :], asle[:, k, :], C["ID128H"][:])
            for k in range(3):
                nm = 3 if k < 2 else 2
                dstp = p_xv[:, cs].rearrange("p (b s j) -> p b s j",
                                             b=8, s=2, j=8)[:, 3 * k:3 * k + nm, :, :]
                srcp = pxtP[:, k, :].rearrange("p (m g s j) -> p m g s j",
                                               m=4, g=2, s=2, j=8)[:, 0:nm, 0, :, :]
                V.tensor_copy(dstp, srcp)
                dsta = araw[:, ss].rearrange("p (b s) -> p b s",
                                             b=8, s=2)[:, 3 * k:3 * k + nm, :]
                srca = altP[:, k, :].rearrange("p (m g) -> p m g",
                                               m=4, g=32)[:, 0:nm, 0:2]
                V.tensor_copy(dsta, srca)

        # ---------------- Phase B (per chunk) ----------------
        CF = FC // NCH          # 1024 fc cols per chunk
        CL = NSLOT // NCH       # 128 slot cols per chunk
        x3 = lambda ap: ap.rearrange("p (c j) -> p c j", j=8)

        for ch in range(NCH):
            fs = slice(CF * ch, CF * ch + CF)
            sl = slice(CL * ch, CL * ch + CL)
            pF, gF = p_xv[:, fs], g_xv[:, fs]
            gtF, ptF, qF = gt_xv[:, fs], pt_xv[:, fs], q_xv[:, fs]
            ztF, mmF = zt_xv[:, fs], mm_xv[:, fs]
            s1F, s2F, s3F = sc1[:, fs], sc2[:, fs], sc3[:, fs]
            lbcF, bvbcF = lbc[:, fs], bvbc[:, fs]
            u32F = u32[:, fs]
            c0L, viL, nviL, inL = c0s[:, sl], viol[:, sl], nviol[:, sl], infs[:, sl]
            nmL, dnL, lmL = nums[:, sl], dens[:, sl], lams[:, sl]
            t1L, t2L = t1s[:, sl], t2s[:, sl]
            arL, a4L = araw[:, sl], alpha4[:, sl]
            bcv = lambda apL: apL.broadcast_to((128, CL, 8))

            # px bias + alpha sigmoid (deferred from phase A)
            V.tensor_tensor(
                pF.rearrange("p (o c) -> p o c", c=128),
                pF.rearrange("p (o c) -> p o c", c=128),
                C["B31X"][:].rearrange("p (o c) -> p o c", o=1)
                .broadcast_to((128, CF // 128, 128)),
                AL.add)
            SC.activation(a4L, arL, AF.Sigmoid, bias=C["B32s"][:])

            # c0 = Lfhx + 4*sigm*(16 - sxx);  (alpha4 holds the sigmoid)
            GP.tensor_scalar(t1L, sxx[:, sl], -1.0, 16.0, AL.mult, AL.add)
            GP.tensor_tensor(t2L, a4L, t1L, AL.mult)
            GP.tensor_scalar(t2L, t2L, 4.0, None, AL.mult)
            GP.tensor_tensor(c0L, t2L, lfhx[:, sl], AL.add)

            # transformed QP data
            SC.sign(s1F, gF)                                   # sigma
            V.tensor_tensor(ptF, s1F, pF, AL.mult)             # pt = sigma*p
            V.tensor_scalar(ztF, ptF, -1.0, None, AL.mult)     # zt0 = -pt
            SC.activation(gtF, gF, AF.Abs)
            SC.activation(qF, gF, AF.Square)
            GP.memset(mmF, 1.0)

            # c(0) and feasibility
            V.tensor_scalar(s2F, ztF, 1.0, -1.0, AL.min, AL.max)   # u0
            V.tensor_tensor(s1F, gtF, s2F, AL.mult)
            V.tensor_reduce(t1L, x3(s1F), XL, AL.add)
            GP.tensor_tensor(t1L, c0L, t1L, AL.add)
            GP.tensor_scalar(viL, t1L, 0.0, None, AL.is_lt)
            GP.tensor_scalar(nviL, viL, -1.0, None, AL.mult)
            V.tensor_reduce(t2L, x3(gtF), XL, AL.add)
            GP.tensor_tensor(t2L, c0L, t2L, AL.add)
            GP.tensor_scalar(inL, t2L, 0.0, None, AL.is_lt)
            GP.tensor_tensor(inL, inL, viL, AL.mult)

            # initial num/den (zt = -pt, mm = 1)
            V.tensor_tensor(s1F, gtF, ztF, AL.mult)
            V.tensor_reduce(nmL, x3(s1F), XL, AL.add)
            GP.tensor_tensor(nmL, c0L, nmL, AL.add)
            V.tensor_reduce(dnL, x3(qF), XL, AL.add)

            def calc_lam():
                GP.tensor_scalar(t1L, dnL, EPS, None, AL.add)
                V.reciprocal(t2L, t1L)
                GP.tensor_tensor(lmL, nmL, t2L, AL.mult)
                GP.tensor_tensor(lmL, lmL, nviL, AL.mult)      # lam = -num/den*viol

            calc_lam()
            for _ in range(T_KIWIEL):
                SC.activation(x3(lbcF), bcv(lmL), AF.Copy)         # lam bcast
                V.tensor_tensor(s2F, lbcF, gtF, AL.mult)
                V.tensor_tensor(s2F, s2F, ptF, AL.subtract)        # ur
                V.tensor_scalar(s2F, s2F, 1.0, -1.0, AL.min, AL.max)  # uhat
                V.tensor_tensor(s1F, gtF, s2F, AL.mult)
                V.tensor_reduce(t1L, x3(s1F), XL, AL.add)
                GP.tensor_tensor(t1L, c0L, t1L, AL.add)            # c
                GP.tensor_scalar(t2L, t1L, -1.0, None, AL.mult)    # -c
                SC.activation(x3(bvbcF), bcv(t2L), AF.Sign)        # bvs = sign(-c)
                V.tensor_tensor(s1F, bvbcF, s2F, AL.mult)
                V.tensor_scalar(s1F, s1F, 1.0, None, AL.is_ge)
                V.tensor_tensor(s1F, s1F, mmF, AL.mult)            # fix
                GP.tensor_tensor(s3F, bvbcF, ztF, AL.subtract)
                V.tensor_tensor(s3F, s1F, s3F, AL.mult)
                V.tensor_tensor(ztF, ztF, s3F, AL.add)
                GP.tensor_tensor(mmF, mmF, s1F, AL.subtract)
                V.tensor_tensor(s1F, gtF, ztF, AL.mult)
                V.tensor_reduce(nmL, x3(s1F), XL, AL.add)
                GP.tensor_tensor(nmL, c0L, nmL, AL.add)
                V.tensor_tensor(s1F, qF, mmF, AL.mult)
                V.tensor_reduce(dnL, x3(s1F), XL, AL.add)
                calc_lam()

            # infeasible rows -> lam = LAMCAP
            GP.tensor_scalar(t1L, lmL, -1.0, LAMCAP, AL.mult, AL.add)
            GP.tensor_tensor(t1L, t1L, inL, AL.mult)
            GP.tensor_tensor(lmL, lmL, t1L, AL.add)
            SC.activation(x3(lbcF), bcv(lmL), AF.Copy)
            V.tensor_tensor(s1F, lbcF, gF, AL.mult)
            V.tensor_tensor(s1F, s1F, pF, AL.subtract)
            V.tensor_scalar(u32F, s1F, 1.0, -1.0, AL.min, AL.max)
            for tt in range(TPC * ch, TPC * (ch + 1)):
                nc.sync.dma_start(
                    u_d[tt * TILE:(tt + 1) * TILE, :].rearrange(
                        "(b r s) j -> r b s j", b=8, r=128, s=2),
                    u32[:, 128 * tt:128 * tt + 128].rearrange(
                        "p (b s j) -> p b s j", b=8, s=2, j=8))


def _build():
    from concourse import bacc, mybir
    from concourse import tile as tile_mod
    from concourse._compat import axon_active
    f32 = mybir.dt.float32
    bf16 = mybir.dt.bfloat16
    nc = bacc.Bacc("TRN2", target_bir_lowering=False,
                   debug=not axon_active(), num_devices=NCORES)
    x_d = nc.dram_tensor("x", [S, N], f32, kind="ExternalInput").ap()
    u_d = nc.dram_tensor("u", [S, N], f32, kind="ExternalOutput").ap()
    cds = {}
    for k, v in _CSHAPES_BF.items():
        cds[k] = nc.dram_tensor(k, list(v), bf16, kind="ExternalInput").ap()
    for k, v in _CSHAPES_F32.items():
        cds[k] = nc.dram_tensor(k, list(v), f32, kind="ExternalInput").ap()
    with tile_mod.TileContext(nc) as tc:
        build_kernel(nc, tc, x_d, u_d, cds)
    nc.compile()
    return nc


def kernel(x, W1, b1, W21, b21, W22, b22, W31, b31, W32, b32, A, G, mean, std):
    from concourse.bass_utils import run_bass_kernel_spmd
    f32 = np.float32
    x = np.asarray(x, f32)
    x0 = (x * np.asarray(std, f32) + np.asarray(mean, f32)).astype(f32)

    consts = _consts(np.asarray(W1, f32), np.asarray(b1, f32), np.asarray(W21, f32),
                     np.asarray(b21, f32), np.asarray(W22, f32), np.asarray(b22, f32),
                     np.asarray(W31, f32), np.asarray(b31, f32), np.asarray(W32, f32),
                     np.asarray(b32, f32), np.asarray(A, f32), np.asarray(G, f32))
    if "nc" not in _CACHE:
        _CACHE["nc"] = _build()
    nc = _CACHE["nc"]

    in_maps = []
    for c in range(NCORES):
        m = {"x": np.ascontiguousarray(x0[c * S:(c + 1) * S])}
        m.update(consts)
        in_maps.append(m)
    res = run_bass_kernel_spmd(nc, in_maps, list(range(NCORES)))
    out = np.concatenate([np.asarray(res.results[c]["u"]) for c in range(NCORES)],
                         axis=0)
    return out.astype(f32)
